# revision 10
# baseline (speedup 1.0000x reference)
"""NT-Xent / SimCLR contrastive loss on 8 Trainium2 NeuronCores (Bass/Tile).

Problem: zi, zj [4096, 512] f32 -> scalar loss.
  reps = concat(zi, zj)            [8192, 512]
  rn   = reps / max(||reps||, 1e-8)
  sim  = rn @ rn.T                 [8192, 8192]
  pos_i  = sim[i, (i+B) mod 2B]
  denom_i = sum_{j != i} exp(sim_ij / tau)
  loss = mean(-pos/tau + log(denom))

Sharding (per the hint: each device holds its row block of normalized reps
plus the full normalized reps for the GEMM): core c owns sim rows
[c*1024, (c+1)*1024). The host normalizes in f32 (identical math to the
reference) and ships the full normalized reps transposed + quantized to
fp8e4m3 (rnT, replicated) and the core's own row-block slice (lhsT). The
device then runs a pure fp8 DoubleRow GEMM -> Exp -> row-reduce pipeline
with nothing on the critical path ahead of the first matmul.

Numerics:
- Diagonal exclusion is exact: sim_ii is extracted from PSUM (identity-mask
  multiply+reduce, f32), passed through the same ACT Exp LUT, and
  subtracted, so the huge exp(sim_ii/tau) term cancels bit-exactly.
- Positives come from the same PSUM via the partner-block diagonal. For
  m-tile t and 2048-col group g, the diagonal of local 128-col blocks t and
  t+8 covers, across g, all 8 possible positions of both the self and the
  partner diagonals (position 2g+half == c resp. (c+4) mod 8); host-side
  one-hot masks select the right candidate per core.
"""

import sys

for _p in ("/opt/trn_rl_repo",):
    if _p not in sys.path:
        sys.path.insert(0, _p)

from contextlib import ExitStack

import ml_dtypes
import numpy as np

TAU = 0.07
B, D = 4096, 512
NCORES = 8
ROWS = 2 * B              # 8192
RPC = ROWS // NCORES      # 1024 rows per core
NM = RPC // 128           # 8 m-tiles per core
KC = D // 128             # 4 contraction chunks
NG = 4                    # column groups
GW = ROWS // NG           # 2048 cols per group
NCAND = 2 * NG            # 8 diag candidates per m-tile

_prog_cache = {}


def _build_program():
    import concourse.bacc as bacc
    import concourse.tile as tile
    import concourse.mybir as mybir

    dt = mybir.dt
    Alu = mybir.AluOpType
    Act = mybir.ActivationFunctionType

    nc = bacc.Bacc("TRN2", target_bir_lowering=False, debug=False,
                   enable_asserts=False, num_devices=NCORES)

    rnT_in = nc.dram_tensor("rnT", [KC, 128, ROWS], dt.float8e4,
                            kind="ExternalInput").ap()
    lhsT_in = nc.dram_tensor("lhsT", [KC, 128, RPC], dt.float8e4,
                             kind="ExternalInput").ap()
    ident_in = nc.dram_tensor("ident_f32", [128, 128], dt.float32,
                              kind="ExternalInput").ap()
    smask_in = nc.dram_tensor("selfmask", [128, NCAND], dt.float32,
                              kind="ExternalInput").ap()
    pmask_in = nc.dram_tensor("posmask", [128, NCAND], dt.float32,
                              kind="ExternalInput").ap()
    out = nc.dram_tensor("out", [128, 2 * NM], dt.float32,
                         kind="ExternalOutput").ap()

    inv_tau = float(1.0 / TAU)

    with tile.TileContext(nc) as tc, ExitStack() as ctx:
        const = ctx.enter_context(tc.tile_pool(name="const", bufs=1))
        persist = ctx.enter_context(tc.tile_pool(name="persist", bufs=1))
        ep = ctx.enter_context(tc.tile_pool(name="ep", bufs=3))
        scrp = ctx.enter_context(tc.tile_pool(name="scrp", bufs=4))
        smallp = ctx.enter_context(tc.tile_pool(name="smallp", bufs=4))
        gpsum = ctx.enter_context(tc.tile_pool(name="gpsum", bufs=2,
                                               space="PSUM"))

        # Dummy activation issued first so the ACT table load (which bacc
        # attaches to the first activation) happens during the DMA prologue
        # instead of stalling the first real Exp.
        warm = const.tile([128, 1], dt.float32, tag="warm")
        nc.gpsimd.memset(warm[:], 0.0)
        warm2 = const.tile([128, 1], dt.float32, tag="warm2")
        nc.scalar.activation(warm2[:], warm[:], Act.Exp, scale=1.0)

        i32 = const.tile([128, 128], dt.float32, tag="i32")
        smask = const.tile([128, NCAND], dt.float32, tag="smask")
        pmask = const.tile([128, NCAND], dt.float32, tag="pmask")

        lhsT = persist.tile([128, KC * RPC], dt.float8e4, tag="lhsT")
        lhsT_v = lhsT[:].rearrange("p (c w) -> p c w", c=KC)
        rnT = persist.tile([128, KC * ROWS], dt.float8e4, tag="rnT")
        rnT_v = rnT[:].rearrange("p (c w) -> p c w", c=KC)

        # Input DMAs ride the sync + gpsimd queues only (both engines are
        # otherwise idle; issuing from scalar would delay the first Exp since
        # engine instruction streams are in-order). The first GEMM unit's
        # dependencies (rnT group 0 chunks 0-1, lhsT chunks 0-1) are issued
        # first so the PE starts as early as possible.
        qs = [nc.sync, nc.gpsimd]
        nq = 0

        def qdma(dst, src):
            nonlocal nq
            qs[nq % len(qs)].dma_start(dst, src)
            nq += 1

        def load_rnT(g, c):
            qdma(rnT_v[:, c, g * GW:(g + 1) * GW],
                 rnT_in[c, :, g * GW:(g + 1) * GW])

        def load_lhsT(c):
            qdma(lhsT_v[:, c, :], lhsT_in[c])

        load_rnT(0, 0)
        load_lhsT(0)
        load_rnT(0, 1)
        load_lhsT(1)
        load_rnT(0, 2)
        load_lhsT(2)
        load_rnT(0, 3)
        load_lhsT(3)
        qdma(i32[:], ident_in[:])
        qdma(smask[:], smask_in[:])
        qdma(pmask[:], pmask_in[:])
        for g in range(1, NG):
            for c in range(KC):
                load_rnT(g, c)

        dvtabs = persist.tile([128, NM * NCAND], dt.float32, tag="dvtabs")
        rstabs = persist.tile([128, NM * NG], dt.float32, tag="rstabs")
        outbuf = persist.tile([128, 2 * NM], dt.float32, tag="outbuf")

        for g in range(NG):
            for t in range(NM):
                ps = gpsum.tile([128, GW], dt.float32, tag="ps")
                for cp in range(2):
                    for h in range(4):
                        nc.tensor.matmul(
                            ps[:, h * 512:(h + 1) * 512],
                            lhsT_v[:, 2 * cp:2 * cp + 2,
                                   t * 128:(t + 1) * 128],
                            rnT_v[:, 2 * cp:2 * cp + 2,
                                  g * GW + h * 512:g * GW + (h + 1) * 512],
                            perf_mode=mybir.MatmulPerfMode.DoubleRow,
                            start=(cp == 0), stop=(cp == 1))
                # diag candidates: local blocks t and t+8 (self or partner
                # diagonal when 2g+half == c resp. (c+4)%8)
                for half in range(2):
                    o = (t + 8 * half) * 128
                    scr = scrp.tile([128, 128], dt.float32, tag="scr128")
                    col = t * NCAND + 2 * g + half
                    nc.vector.scalar_tensor_tensor(
                        out=scr[:], in0=ps[:, o:o + 128], scalar=1.0,
                        in1=i32[:], op0=Alu.mult, op1=Alu.mult,
                        accum_out=dvtabs[:, col:col + 1])
                e = ep.tile([128, GW], dt.bfloat16, tag="e")
                nc.scalar.activation(e[:], ps[:], Act.Exp, scale=inv_tau)
                # row-sum on DVE (frees ACT from the accumulator read); the
                # self term is bf16-rounded here and identically in the
                # selfexp path below, keeping the cancellation bit-exact
                nc.vector.reduce_sum(
                    rstabs[:, t * NG + g:t * NG + g + 1], e[:],
                    axis=mybir.AxisListType.X)
                if g == NG - 1:
                    # epilogue for m-tile t, emitted inline so it overlaps
                    # the remaining units' exps instead of queuing after them
                    scr8 = smallp.tile([128, NCAND], dt.float32, tag="scr8")
                    selfsim = smallp.tile([128, 1], dt.float32,
                                          tag="selfsim")
                    nc.vector.scalar_tensor_tensor(
                        out=scr8[:],
                        in0=dvtabs[:, t * NCAND:(t + 1) * NCAND],
                        scalar=1.0, in1=smask[:], op0=Alu.mult,
                        op1=Alu.mult, accum_out=selfsim[:])
                    scr8b = smallp.tile([128, NCAND], dt.float32,
                                        tag="scr8b")
                    nc.vector.scalar_tensor_tensor(
                        out=scr8b[:],
                        in0=dvtabs[:, t * NCAND:(t + 1) * NCAND],
                        scalar=1.0, in1=pmask[:], op0=Alu.mult,
                        op1=Alu.mult,
                        accum_out=outbuf[:, NM + t:NM + t + 1])
                    selfexp = smallp.tile([128, 1], dt.bfloat16,
                                          tag="selfexp")
                    nc.scalar.activation(selfexp[:], selfsim[:], Act.Exp,
                                         scale=inv_tau)
                    rowsum = smallp.tile([128, 1], dt.float32, tag="rowsum")
                    nc.vector.reduce_sum(
                        rowsum[:], rstabs[:, t * NG:(t + 1) * NG],
                        axis=mybir.AxisListType.X)
                    nc.vector.tensor_sub(outbuf[:, t:t + 1], rowsum[:],
                                         selfexp[:])

        nc.sync.dma_start(out[:], outbuf[:])

    # Restrict bacc's activation-table choices to the one table that holds
    # Exp+Copy together, so exactly one ACT table load is emitted.
    import concourse.bacc as bacc_mod
    _orig_tables = bacc_mod.get_activation_tables

    def _only_lnexp(arch):
        keep = "natural_log_exp_and_others"
        return {k: (v if k == keep else set())
                for k, v in _orig_tables(arch).items()}

    bacc_mod.get_activation_tables = _only_lnexp
    try:
        nc.compile()
    finally:
        bacc_mod.get_activation_tables = _orig_tables
    return nc


def _host_inputs(zi, zj):
    reps = np.concatenate([np.asarray(zi, np.float32),
                           np.asarray(zj, np.float32)], axis=0)
    norms = np.maximum(np.sqrt((reps * reps).sum(axis=1, keepdims=True)),
                       1e-8)
    rn8 = (reps / norms).astype(ml_dtypes.float8_e4m3fn)        # [2B, D]
    rnT = np.ascontiguousarray(rn8.T.reshape(KC, 128, ROWS))
    ident_f32 = np.eye(128, dtype=np.float32)
    in_maps = []
    for c in range(NCORES):
        lhsT = np.ascontiguousarray(
            rn8[c * RPC:(c + 1) * RPC].T.reshape(KC, 128, RPC))
        smask = np.zeros((128, NCAND), np.float32)
        smask[:, c] = 1.0
        pmask = np.zeros((128, NCAND), np.float32)
        pmask[:, (c + 4) % 8] = 1.0
        in_maps.append({
            "rnT": rnT, "lhsT": lhsT, "ident_f32": ident_f32,
            "selfmask": smask, "posmask": pmask,
        })
    return in_maps


def _postprocess(results):
    denom = np.empty((ROWS,), np.float64)
    pos = np.empty((ROWS,), np.float64)
    for c in range(NCORES):
        o = np.asarray(results[c]["out"], np.float64)  # [128, 16]
        for t in range(NM):
            rows = slice(c * RPC + t * 128, c * RPC + (t + 1) * 128)
            denom[rows] = o[:, t]
            pos[rows] = o[:, NM + t]
    loss = np.mean(-pos / TAU + np.log(denom))
    return np.asarray(loss, dtype=np.float32)


def kernel(zi, zj, _trace=False):
    from concourse.bass_utils import run_bass_kernel_spmd

    if "nc" not in _prog_cache:
        _prog_cache["nc"] = _build_program()
    nc = _prog_cache["nc"]
    in_maps = _host_inputs(zi, zj)
    res = run_bass_kernel_spmd(nc, in_maps, list(range(NCORES)),
                               trace=_trace)
    _prog_cache["last_result"] = res
    return _postprocess(res.results)


# revision 12
# speedup vs baseline: 1.3294x; 1.3294x over previous
"""NT-Xent / SimCLR contrastive loss on 8 Trainium2 NeuronCores (Bass/Tile).

Problem: zi, zj [4096, 512] f32 -> scalar loss.
  reps = concat(zi, zj)            [8192, 512]
  rn   = reps / max(||reps||, 1e-8)
  sim  = rn @ rn.T                 [8192, 8192]
  pos_i  = sim[i, (i+B) mod 2B]
  denom_i = sum_{j != i} exp(sim_ij / tau)
  loss = mean(-pos/tau + log(denom))

Sharding (per the hint: each device holds its row block of normalized reps
plus the full normalized reps for the GEMM): core c owns sim rows
[c*1024, (c+1)*1024). The host normalizes in f32 (identical math to the
reference) and ships the full normalized reps transposed + quantized to
fp8e4m3 (rnT, replicated) and the core's own row-block slice (lhsT). The
device then runs a pure fp8 DoubleRow GEMM -> Exp -> row-reduce pipeline
with nothing on the critical path ahead of the first matmul.

Numerics:
- Diagonal exclusion is exact: sim_ii is extracted from PSUM (identity-mask
  multiply+reduce, f32), passed through the same ACT Exp LUT, and
  subtracted, so the huge exp(sim_ii/tau) term cancels bit-exactly.
- Positives come from the same PSUM via the partner-block diagonal. For
  m-tile t and 2048-col group g, the diagonal of local 128-col blocks t and
  t+8 covers, across g, all 8 possible positions of both the self and the
  partner diagonals (position 2g+half == c resp. (c+4) mod 8); host-side
  one-hot masks select the right candidate per core.
"""

import sys

for _p in ("/opt/trn_rl_repo",):
    if _p not in sys.path:
        sys.path.insert(0, _p)

from contextlib import ExitStack

import ml_dtypes
import numpy as np

TAU = 0.07
B, D = 4096, 512
NCORES = 8
ROWS = 2 * B              # 8192
RPC = ROWS // NCORES      # 1024 rows per core
NM = RPC // 128           # 8 m-tiles per core
KC = D // 128             # 4 contraction chunks
NG = 4                    # column groups
GW = ROWS // NG           # 2048 cols per group
NCAND = 2 * NG            # 8 diag candidates per m-tile

_prog_cache = {}


def _build_program():
    import concourse.bacc as bacc
    import concourse.tile as tile
    import concourse.mybir as mybir

    dt = mybir.dt
    Alu = mybir.AluOpType
    Act = mybir.ActivationFunctionType

    nc = bacc.Bacc("TRN2", target_bir_lowering=False, debug=False,
                   enable_asserts=False, num_devices=NCORES)

    rnT_in = nc.dram_tensor("rnT", [KC, 128, ROWS], dt.float8e4,
                            kind="ExternalInput").ap()
    lhsT_in = nc.dram_tensor("lhsT", [KC, 128, RPC], dt.float8e4,
                             kind="ExternalInput").ap()
    ident_in = nc.dram_tensor("ident_f32", [128, 128], dt.float32,
                              kind="ExternalInput").ap()
    smask_in = nc.dram_tensor("selfmask", [128, NCAND], dt.float32,
                              kind="ExternalInput").ap()
    pmask_in = nc.dram_tensor("posmask", [128, NCAND], dt.float32,
                              kind="ExternalInput").ap()
    out = nc.dram_tensor("out", [128, 2 * NM], dt.float32,
                         kind="ExternalOutput").ap()

    inv_tau = float(1.0 / TAU)

    with tile.TileContext(nc) as tc, ExitStack() as ctx:
        const = ctx.enter_context(tc.tile_pool(name="const", bufs=1))
        persist = ctx.enter_context(tc.tile_pool(name="persist", bufs=1))
        ep = ctx.enter_context(tc.tile_pool(name="ep", bufs=3))
        scrp = ctx.enter_context(tc.tile_pool(name="scrp", bufs=4))
        smallp = ctx.enter_context(tc.tile_pool(name="smallp", bufs=4))
        gpsum = ctx.enter_context(tc.tile_pool(name="gpsum", bufs=2,
                                               space="PSUM"))

        # Dummy activation issued first so the ACT table load (which bacc
        # attaches to the first activation) happens during the DMA prologue
        # instead of stalling the first real Exp.
        warm = const.tile([128, 1], dt.float32, tag="warm")
        nc.gpsimd.memset(warm[:], 0.0)
        warm2 = const.tile([128, 1], dt.float32, tag="warm2")
        nc.scalar.activation(warm2[:], warm[:], Act.Exp, scale=1.0)

        i32 = const.tile([128, 128], dt.float32, tag="i32")
        smask = const.tile([128, NCAND], dt.float32, tag="smask")
        pmask = const.tile([128, NCAND], dt.float32, tag="pmask")

        lhsT = persist.tile([128, KC * RPC], dt.float8e4, tag="lhsT")
        lhsT_v = lhsT[:].rearrange("p (c w) -> p c w", c=KC)
        rnT = persist.tile([128, KC * ROWS], dt.float8e4, tag="rnT")
        rnT_v = rnT[:].rearrange("p (c w) -> p c w", c=KC)

        # Input DMAs ride the sync + gpsimd queues only (both engines are
        # otherwise idle; issuing from scalar would delay the first Exp since
        # engine instruction streams are in-order). The first GEMM unit's
        # dependencies (rnT group 0 chunks 0-1, lhsT chunks 0-1) are issued
        # first so the PE starts as early as possible.
        qs = [nc.sync, nc.gpsimd]
        nq = 0

        def qdma(dst, src):
            nonlocal nq
            qs[nq % len(qs)].dma_start(dst, src)
            nq += 1

        def load_rnT(g, c):
            qdma(rnT_v[:, c, g * GW:(g + 1) * GW],
                 rnT_in[c, :, g * GW:(g + 1) * GW])

        def load_lhsT(c):
            qdma(lhsT_v[:, c, :], lhsT_in[c])

        load_rnT(0, 0)
        load_lhsT(0)
        load_rnT(0, 1)
        load_lhsT(1)
        load_rnT(0, 2)
        load_lhsT(2)
        load_rnT(0, 3)
        load_lhsT(3)
        qdma(i32[:], ident_in[:])
        qdma(smask[:], smask_in[:])
        qdma(pmask[:], pmask_in[:])
        for g in range(1, NG):
            for c in range(KC):
                load_rnT(g, c)

        dvtabs = persist.tile([128, NM * NCAND], dt.float32, tag="dvtabs")
        rstabs = persist.tile([128, NM * NG], dt.float32, tag="rstabs")
        outbuf = persist.tile([128, 2 * NM], dt.float32, tag="outbuf")

        for g in range(NG):
            for t in range(NM):
                ps = gpsum.tile([128, GW], dt.float32, tag="ps")
                for cp in range(2):
                    for h in range(4):
                        nc.tensor.matmul(
                            ps[:, h * 512:(h + 1) * 512],
                            lhsT_v[:, 2 * cp:2 * cp + 2,
                                   t * 128:(t + 1) * 128],
                            rnT_v[:, 2 * cp:2 * cp + 2,
                                  g * GW + h * 512:g * GW + (h + 1) * 512],
                            perf_mode=mybir.MatmulPerfMode.DoubleRow,
                            start=(cp == 0), stop=(cp == 1))
                # diag candidates: local blocks t and t+8 (self or partner
                # diagonal when 2g+half == c resp. (c+4)%8)
                for half in range(2):
                    o = (t + 8 * half) * 128
                    scr = scrp.tile([128, 128], dt.float32, tag="scr128")
                    col = t * NCAND + 2 * g + half
                    nc.vector.scalar_tensor_tensor(
                        out=scr[:], in0=ps[:, o:o + 128], scalar=1.0,
                        in1=i32[:], op0=Alu.mult, op1=Alu.mult,
                        accum_out=dvtabs[:, col:col + 1])
                e = ep.tile([128, GW], dt.bfloat16, tag="e")
                nc.scalar.activation(
                    e[:], ps[:], Act.Exp, scale=inv_tau,
                    accum_out=rstabs[:, t * NG + g:t * NG + g + 1])
                if g == NG - 1:
                    # epilogue for m-tile t, emitted inline so it overlaps
                    # the remaining units' exps instead of queuing after them
                    scr8 = smallp.tile([128, NCAND], dt.float32, tag="scr8")
                    selfsim = smallp.tile([128, 1], dt.float32,
                                          tag="selfsim")
                    nc.vector.scalar_tensor_tensor(
                        out=scr8[:],
                        in0=dvtabs[:, t * NCAND:(t + 1) * NCAND],
                        scalar=1.0, in1=smask[:], op0=Alu.mult,
                        op1=Alu.mult, accum_out=selfsim[:])
                    scr8b = smallp.tile([128, NCAND], dt.float32,
                                        tag="scr8b")
                    nc.vector.scalar_tensor_tensor(
                        out=scr8b[:],
                        in0=dvtabs[:, t * NCAND:(t + 1) * NCAND],
                        scalar=1.0, in1=pmask[:], op0=Alu.mult,
                        op1=Alu.mult,
                        accum_out=outbuf[:, NM + t:NM + t + 1])
                    selfexp = smallp.tile([128, 1], dt.float32,
                                          tag="selfexp")
                    nc.scalar.activation(selfexp[:], selfsim[:], Act.Exp,
                                         scale=inv_tau)
                    rowsum = smallp.tile([128, 1], dt.float32, tag="rowsum")
                    nc.vector.reduce_sum(
                        rowsum[:], rstabs[:, t * NG:(t + 1) * NG],
                        axis=mybir.AxisListType.X)
                    nc.vector.tensor_sub(outbuf[:, t:t + 1], rowsum[:],
                                         selfexp[:])

        nc.sync.dma_start(out[:], outbuf[:])

    # Restrict bacc's activation-table choices to the one table that holds
    # Exp+Copy together, so exactly one ACT table load is emitted.
    import concourse.bacc as bacc_mod
    _orig_tables = bacc_mod.get_activation_tables

    def _only_lnexp(arch):
        keep = "natural_log_exp_and_others"
        return {k: (v if k == keep else set())
                for k, v in _orig_tables(arch).items()}

    bacc_mod.get_activation_tables = _only_lnexp
    try:
        nc.compile()
    finally:
        bacc_mod.get_activation_tables = _orig_tables
    return nc


def _host_inputs(zi, zj):
    reps = np.concatenate([np.asarray(zi, np.float32),
                           np.asarray(zj, np.float32)], axis=0)
    norms = np.maximum(np.sqrt((reps * reps).sum(axis=1, keepdims=True)),
                       1e-8)
    rn8 = (reps / norms).astype(ml_dtypes.float8_e4m3fn)        # [2B, D]
    rnT = np.ascontiguousarray(rn8.T.reshape(KC, 128, ROWS))
    ident_f32 = np.eye(128, dtype=np.float32)
    in_maps = []
    for c in range(NCORES):
        lhsT = np.ascontiguousarray(
            rn8[c * RPC:(c + 1) * RPC].T.reshape(KC, 128, RPC))
        smask = np.zeros((128, NCAND), np.float32)
        smask[:, c] = 1.0
        pmask = np.zeros((128, NCAND), np.float32)
        pmask[:, (c + 4) % 8] = 1.0
        in_maps.append({
            "rnT": rnT, "lhsT": lhsT, "ident_f32": ident_f32,
            "selfmask": smask, "posmask": pmask,
        })
    return in_maps


def _postprocess(results):
    denom = np.empty((ROWS,), np.float64)
    pos = np.empty((ROWS,), np.float64)
    for c in range(NCORES):
        o = np.asarray(results[c]["out"], np.float64)  # [128, 16]
        for t in range(NM):
            rows = slice(c * RPC + t * 128, c * RPC + (t + 1) * 128)
            denom[rows] = o[:, t]
            pos[rows] = o[:, NM + t]
    loss = np.mean(-pos / TAU + np.log(denom))
    return np.asarray(loss, dtype=np.float32)


def kernel(zi, zj, _trace=False):
    from concourse.bass_utils import run_bass_kernel_spmd

    if "nc" not in _prog_cache:
        _prog_cache["nc"] = _build_program()
    nc = _prog_cache["nc"]
    in_maps = _host_inputs(zi, zj)
    res = run_bass_kernel_spmd(nc, in_maps, list(range(NCORES)),
                               trace=_trace)
    _prog_cache["last_result"] = res
    return _postprocess(res.results)


# revision 13
# speedup vs baseline: 1.3507x; 1.0160x over previous
"""NT-Xent / SimCLR contrastive loss on 8 Trainium2 NeuronCores (Bass/Tile).

Problem: zi, zj [4096, 512] f32 -> scalar loss.
  reps = concat(zi, zj)            [8192, 512]
  rn   = reps / max(||reps||, 1e-8)
  sim  = rn @ rn.T                 [8192, 8192]
  pos_i  = sim[i, (i+B) mod 2B]
  denom_i = sum_{j != i} exp(sim_ij / tau)
  loss = mean(-pos/tau + log(denom))

Sharding (per the hint: each device holds its row block of normalized reps
plus the full normalized reps for the GEMM): core c owns sim rows
[c*1024, (c+1)*1024). The host normalizes in f32 (identical math to the
reference) and ships the full normalized reps transposed + quantized to
fp8e4m3 (rnT, replicated) and the core's own row-block slice (lhsT). The
device then runs a pure fp8 DoubleRow GEMM -> Exp -> row-reduce pipeline
with nothing on the critical path ahead of the first matmul.

Numerics:
- Diagonal exclusion is exact: sim_ii is extracted from PSUM (identity-mask
  multiply+reduce, f32), passed through the same ACT Exp LUT, and
  subtracted, so the huge exp(sim_ii/tau) term cancels bit-exactly.
- Positives come from the same PSUM via the partner-block diagonal. For
  m-tile t and 2048-col group g, the diagonal of local 128-col blocks t and
  t+8 covers, across g, all 8 possible positions of both the self and the
  partner diagonals (position 2g+half == c resp. (c+4) mod 8); host-side
  one-hot masks select the right candidate per core.
"""

import sys

for _p in ("/opt/trn_rl_repo",):
    if _p not in sys.path:
        sys.path.insert(0, _p)

from contextlib import ExitStack

import ml_dtypes
import numpy as np

TAU = 0.07
B, D = 4096, 512
NCORES = 8
ROWS = 2 * B              # 8192
RPC = ROWS // NCORES      # 1024 rows per core
NM = RPC // 128           # 8 m-tiles per core
KC = D // 128             # 4 contraction chunks
NG = 4                    # column groups
GW = ROWS // NG           # 2048 cols per group
NCAND = 2 * NG            # 8 diag candidates per m-tile

_prog_cache = {}


def _build_program():
    import concourse.bacc as bacc
    import concourse.tile as tile
    import concourse.mybir as mybir

    dt = mybir.dt
    Alu = mybir.AluOpType
    Act = mybir.ActivationFunctionType

    nc = bacc.Bacc("TRN2", target_bir_lowering=False, debug=False,
                   enable_asserts=False, num_devices=NCORES)

    rnT_in = nc.dram_tensor("rnT", [KC, 128, ROWS], dt.float8e4,
                            kind="ExternalInput").ap()
    lhsT_in = nc.dram_tensor("lhsT", [KC, 128, RPC], dt.float8e4,
                             kind="ExternalInput").ap()
    ident_in = nc.dram_tensor("ident_f32", [128, 128], dt.float32,
                              kind="ExternalInput").ap()
    smask_in = nc.dram_tensor("selfmask", [128, NCAND], dt.float32,
                              kind="ExternalInput").ap()
    pmask_in = nc.dram_tensor("posmask", [128, NCAND], dt.float32,
                              kind="ExternalInput").ap()
    out = nc.dram_tensor("out", [128, 2 * NM], dt.float32,
                         kind="ExternalOutput").ap()

    inv_tau = float(1.0 / TAU)

    with tile.TileContext(nc) as tc, ExitStack() as ctx:
        const = ctx.enter_context(tc.tile_pool(name="const", bufs=1))
        persist = ctx.enter_context(tc.tile_pool(name="persist", bufs=1))
        ep = ctx.enter_context(tc.tile_pool(name="ep", bufs=3))
        scrp = ctx.enter_context(tc.tile_pool(name="scrp", bufs=4))
        smallp = ctx.enter_context(tc.tile_pool(name="smallp", bufs=4))
        gpsum = ctx.enter_context(tc.tile_pool(name="gpsum", bufs=2,
                                               space="PSUM"))

        # Dummy activation issued first so the ACT table load (which bacc
        # attaches to the first activation) happens during the DMA prologue
        # instead of stalling the first real Exp.
        warm = const.tile([128, 1], dt.float32, tag="warm")
        nc.gpsimd.memset(warm[:], 0.0)
        warm2 = const.tile([128, 1], dt.float32, tag="warm2")
        nc.scalar.activation(warm2[:], warm[:], Act.Exp, scale=1.0)

        i32 = const.tile([128, 128], dt.float32, tag="i32")
        smask = const.tile([128, NCAND], dt.float32, tag="smask")
        pmask = const.tile([128, NCAND], dt.float32, tag="pmask")

        lhsT = persist.tile([128, KC * RPC], dt.float8e4, tag="lhsT")
        lhsT_v = lhsT[:].rearrange("p (c w) -> p c w", c=KC)
        rnT = persist.tile([128, KC * ROWS], dt.float8e4, tag="rnT")
        rnT_v = rnT[:].rearrange("p (c w) -> p c w", c=KC)

        # Input DMAs ride the sync + gpsimd queues only (both engines are
        # otherwise idle; issuing from scalar would delay the first Exp since
        # engine instruction streams are in-order). The first GEMM unit's
        # dependencies (rnT group 0 chunks 0-1, lhsT chunks 0-1) are issued
        # first so the PE starts as early as possible.
        qs = [nc.sync, nc.gpsimd]
        nq = 0

        def qdma(dst, src):
            nonlocal nq
            qs[nq % len(qs)].dma_start(dst, src)
            nq += 1

        def load_rnT(g, c):
            qdma(rnT_v[:, c, g * GW:(g + 1) * GW],
                 rnT_in[c, :, g * GW:(g + 1) * GW])

        def load_lhsT(c):
            qdma(lhsT_v[:, c, :], lhsT_in[c])

        # group 0 in 512-col slices, h-major, so the first matmuls' operand
        # slices land as early as possible
        load_lhsT(0)
        load_lhsT(1)
        for h in range(4):
            for c in range(KC):
                lo = h * 512
                qdma(rnT_v[:, c, lo:lo + 512], rnT_in[c, :, lo:lo + 512])
            if h == 0:
                load_lhsT(2)
                load_lhsT(3)
        qdma(i32[:], ident_in[:])
        qdma(smask[:], smask_in[:])
        qdma(pmask[:], pmask_in[:])
        for g in range(1, NG):
            for c in range(KC):
                load_rnT(g, c)

        dvtabs = persist.tile([128, NM * NCAND], dt.float32, tag="dvtabs")
        rstabs = persist.tile([128, NM * NG], dt.float32, tag="rstabs")
        outbuf = persist.tile([128, 2 * NM], dt.float32, tag="outbuf")

        for g in range(NG):
            for t in range(NM):
                ps = gpsum.tile([128, GW], dt.float32, tag="ps")
                for cp in range(2):
                    for h in range(4):
                        nc.tensor.matmul(
                            ps[:, h * 512:(h + 1) * 512],
                            lhsT_v[:, 2 * cp:2 * cp + 2,
                                   t * 128:(t + 1) * 128],
                            rnT_v[:, 2 * cp:2 * cp + 2,
                                  g * GW + h * 512:g * GW + (h + 1) * 512],
                            perf_mode=mybir.MatmulPerfMode.DoubleRow,
                            start=(cp == 0), stop=(cp == 1))
                # diag candidates: local blocks t and t+8 (self or partner
                # diagonal when 2g+half == c resp. (c+4)%8)
                for half in range(2):
                    o = (t + 8 * half) * 128
                    scr = scrp.tile([128, 128], dt.float32, tag="scr128")
                    col = t * NCAND + 2 * g + half
                    nc.vector.scalar_tensor_tensor(
                        out=scr[:], in0=ps[:, o:o + 128], scalar=1.0,
                        in1=i32[:], op0=Alu.mult, op1=Alu.mult,
                        accum_out=dvtabs[:, col:col + 1])
                e = ep.tile([128, GW], dt.bfloat16, tag="e")
                nc.scalar.activation(
                    e[:], ps[:], Act.Exp, scale=inv_tau,
                    accum_out=rstabs[:, t * NG + g:t * NG + g + 1])
                if g == NG - 1:
                    # epilogue for m-tile t, emitted inline so it overlaps
                    # the remaining units' exps instead of queuing after them
                    scr8 = smallp.tile([128, NCAND], dt.float32, tag="scr8")
                    selfsim = smallp.tile([128, 1], dt.float32,
                                          tag="selfsim")
                    nc.vector.scalar_tensor_tensor(
                        out=scr8[:],
                        in0=dvtabs[:, t * NCAND:(t + 1) * NCAND],
                        scalar=1.0, in1=smask[:], op0=Alu.mult,
                        op1=Alu.mult, accum_out=selfsim[:])
                    scr8b = smallp.tile([128, NCAND], dt.float32,
                                        tag="scr8b")
                    nc.vector.scalar_tensor_tensor(
                        out=scr8b[:],
                        in0=dvtabs[:, t * NCAND:(t + 1) * NCAND],
                        scalar=1.0, in1=pmask[:], op0=Alu.mult,
                        op1=Alu.mult,
                        accum_out=outbuf[:, NM + t:NM + t + 1])
                    selfexp = smallp.tile([128, 1], dt.float32,
                                          tag="selfexp")
                    nc.scalar.activation(selfexp[:], selfsim[:], Act.Exp,
                                         scale=inv_tau)
                    rowsum = smallp.tile([128, 1], dt.float32, tag="rowsum")
                    nc.vector.reduce_sum(
                        rowsum[:], rstabs[:, t * NG:(t + 1) * NG],
                        axis=mybir.AxisListType.X)
                    nc.vector.tensor_sub(outbuf[:, t:t + 1], rowsum[:],
                                         selfexp[:])

        nc.sync.dma_start(out[:], outbuf[:])

    # Restrict bacc's activation-table choices to the one table that holds
    # Exp+Copy together, so exactly one ACT table load is emitted.
    import concourse.bacc as bacc_mod
    _orig_tables = bacc_mod.get_activation_tables

    def _only_lnexp(arch):
        keep = "natural_log_exp_and_others"
        return {k: (v if k == keep else set())
                for k, v in _orig_tables(arch).items()}

    bacc_mod.get_activation_tables = _only_lnexp
    try:
        nc.compile()
    finally:
        bacc_mod.get_activation_tables = _orig_tables
    return nc


def _host_inputs(zi, zj):
    reps = np.concatenate([np.asarray(zi, np.float32),
                           np.asarray(zj, np.float32)], axis=0)
    norms = np.maximum(np.sqrt((reps * reps).sum(axis=1, keepdims=True)),
                       1e-8)
    rn8 = (reps / norms).astype(ml_dtypes.float8_e4m3fn)        # [2B, D]
    rnT = np.ascontiguousarray(rn8.T.reshape(KC, 128, ROWS))
    ident_f32 = np.eye(128, dtype=np.float32)
    in_maps = []
    for c in range(NCORES):
        lhsT = np.ascontiguousarray(
            rn8[c * RPC:(c + 1) * RPC].T.reshape(KC, 128, RPC))
        smask = np.zeros((128, NCAND), np.float32)
        smask[:, c] = 1.0
        pmask = np.zeros((128, NCAND), np.float32)
        pmask[:, (c + 4) % 8] = 1.0
        in_maps.append({
            "rnT": rnT, "lhsT": lhsT, "ident_f32": ident_f32,
            "selfmask": smask, "posmask": pmask,
        })
    return in_maps


def _postprocess(results):
    denom = np.empty((ROWS,), np.float64)
    pos = np.empty((ROWS,), np.float64)
    for c in range(NCORES):
        o = np.asarray(results[c]["out"], np.float64)  # [128, 16]
        for t in range(NM):
            rows = slice(c * RPC + t * 128, c * RPC + (t + 1) * 128)
            denom[rows] = o[:, t]
            pos[rows] = o[:, NM + t]
    loss = np.mean(-pos / TAU + np.log(denom))
    return np.asarray(loss, dtype=np.float32)


def kernel(zi, zj, _trace=False):
    from concourse.bass_utils import run_bass_kernel_spmd

    if "nc" not in _prog_cache:
        _prog_cache["nc"] = _build_program()
    nc = _prog_cache["nc"]
    in_maps = _host_inputs(zi, zj)
    res = run_bass_kernel_spmd(nc, in_maps, list(range(NCORES)),
                               trace=_trace)
    _prog_cache["last_result"] = res
    return _postprocess(res.results)
